# revision 2
# baseline (speedup 1.0000x reference)
"""DropConnect forward kernel v2 for Trainium2 (8 NeuronCores, Bass/Tile).

y[n,o] = (sum_k x[n,k] * weight[k,o] * w_mask[n,k,o] + bias[o]*b_mask[n,o]) * 2

Data-parallel over batch N=256 -> 32 samples/core. The dominant cost is
streaming w_mask (256M elements); the 0/1 values are stored sub-bf16:
  - cols [0,   C0)  as bf16  (exact)   -> DVE tensor_mul at 2x (bf16)
  - cols [C0,  C1)  as fp8e4 (exact)   -> ACT upcasts to bf16, DVE muls at 2x
  - cols [C1, 8192) as fp8e4 (exact)   -> GPSIMD (Pool) tensor_mul directly
This splits the mask x (2*weight) elementwise product across all three
non-PE compute engines while cutting mask DMA from 2 B/elem to ~1.27 B/elem.

Reduction over k uses the PE with stationary = 128x128 product tiles and
moving = one x column per (sample, j): out[o_tile, 1] accumulates over j
in PSUM, so each sample's output lands partition-dense as [1024 o, 1 n]
(8 o-tile columns in a [128, 8] PSUM tile). The epilogue adds the
host-precomputed 2*bias*b_mask and the full output leaves as one
[128, 256] f32 tile per core, n-major; the host un-transposes (free).

Layout: k = 8p + j (p = SBUF partition, j = 0..7), mask free index
f = j*1024 + o; x is staged as xt[p, j*32+n] = x[n, 8p+j] in bf16.
"""

import sys

for _p in ("/opt/trn_rl_repo",):
    if _p not in sys.path:
        sys.path.insert(0, _p)

import numpy as np

import concourse.bass as bass
import concourse.tile as tile
from concourse import bacc, mybir
from concourse.bass_utils import run_bass_kernel_spmd

N_CORES = 8
NS = 32            # samples per core
D = 1024           # in_dim == out_dim
P = 128            # SBUF partitions
J = D // P         # 8 k-subtiles interleaved per partition row
F = J * D          # 8192 free elements per mask slab
OT = D // P        # 8 o-tiles of 128

# region split (multiples of 128): [0,C0) bf16 -> DVE TT directly,
# [C0,F) fp8 -> ACT upcasts to bf16, DVE TT (GPSIMD unused: its SBUF
# traffic knocks the DVE out of the 2x perf mode, measured)
C0 = 3072

FP32 = mybir.dt.float32
BF16 = mybir.dt.bfloat16
FP8 = mybir.dt.float8e4

# test.py pokes this to get a traced run; the grading path never touches it.
TRACE = {"trace": False, "last_result": None, "trace_kwargs": {}}


def _build_nc(ns: int = NS):
    nc = bacc.Bacc("TRN2", target_bir_lowering=False, debug=False)

    wm16 = nc.declare_dram_parameter("wm16", [ns, P, C0], BF16, isOutput=False)
    wm8 = nc.declare_dram_parameter("wm8", [ns, P, F - C0], FP8, isOutput=False)
    wp = nc.declare_dram_parameter("wp", [P, F], BF16, isOutput=False)
    xt = nc.declare_dram_parameter("xt", [P, J * ns], BF16, isOutput=False)
    bb = nc.declare_dram_parameter("bb", [P, OT * ns], FP32, isOutput=False)
    y = nc.declare_dram_parameter("y", [P, OT * ns], FP32, isOutput=True)

    with tile.TileContext(nc) as tc:
        with (
            tc.tile_pool(name="const", bufs=1) as cpool,
            tc.tile_pool(name="s16", bufs=4) as spool16,
            tc.tile_pool(name="s8", bufs=4) as spool8,
            tc.tile_pool(name="up", bufs=3) as uppool,
            tc.tile_pool(name="prA", bufs=3) as prpoolA,
            tc.tile_pool(name="prB", bufs=3) as prpoolB,
            tc.tile_pool(name="psum", bufs=4, space=bass.MemorySpace.PSUM) as ppool,
        ):
            # constants on the ACT ring: the mask slabs (sync ring) start
            # streaming at t=0 concurrently. The GpSimd ring is never used:
            # even its SWDGE descriptor generation (on the Q7 cores) knocks
            # the DVE out of the 2x perf mode.
            wpt = []
            for lo, hi in ((0, C0), (C0, F)):
                t = cpool.tile([P, hi - lo], BF16, tag=f"wp{lo}")
                nc.scalar.dma_start(out=t[:], in_=wp[:, lo:hi])
                wpt.append(t)
            xtt = cpool.tile([P, J * ns], BF16, tag="xt")
            nc.scalar.dma_start(out=xtt[:], in_=xt[:])
            bbt = cpool.tile([P, OT * ns], FP32, tag="bb")
            nc.scalar.dma_start(out=bbt[:], in_=bb[:])
            yt = cpool.tile([P, OT * ns], FP32, tag="y")

            # per-sample epilogues drained two samples late so the PE tail
            # never stalls DVE; the first y half leaves as soon as its
            # epilogues are done
            pending = []  # (sample, ps)

            def drain_epi():
                m, psm = pending.pop(0)
                nc.vector.tensor_add(
                    yt[:, m * OT : (m + 1) * OT],
                    psm[:],
                    bbt[:, m * OT : (m + 1) * OT],
                )
            for n in range(ns):
                s8 = spool8.tile([P, F - C0], FP8, tag="s8")
                s16 = spool16.tile([P, C0], BF16, tag="s16")
                if n == 0:  # pA(0) is the first DVE op: its slab first
                    nc.sync.dma_start(out=s16[:], in_=wm16[n, :, :])
                    nc.sync.dma_start(out=s8[:], in_=wm8[n, :, :])
                else:
                    nc.sync.dma_start(out=s8[:], in_=wm8[n, :, :])
                    nc.sync.dma_start(out=s16[:], in_=wm16[n, :, :])
                # products: ACT upcasts the fp8 region, DVE multiplies both
                up = uppool.tile([P, F - C0], BF16, tag="up")
                nc.scalar.copy(up[:], s8[:])
                pA = prpoolA.tile([P, C0], BF16, tag="pA")
                nc.vector.tensor_mul(pA[:], s16[:], wpt[0][:])
                if len(pending) >= 2:
                    drain_epi()
                pB = prpoolB.tile([P, F - C0], BF16, tag="pB")
                nc.vector.tensor_mul(pB[:], up[:], wpt[1][:])

                # PE: 8 o-tiles x 8 j-steps, accumulate over j in PSUM
                ps = ppool.tile([P, OT], FP32, tag="ps")
                for ot in range(OT):
                    for j in range(J):
                        c = j * D + ot * P
                        if c < C0:
                            lhsT = pA[:, c : c + P]
                        else:
                            lhsT = pB[:, c - C0 : c - C0 + P]
                        nc.tensor.matmul(
                            ps[:, ot : ot + 1],
                            lhsT,
                            xtt[:, j * ns + n : j * ns + n + 1],
                            start=(j == 0),
                            stop=(j == J - 1),
                        )
                pending.append((n, ps))

            while pending:
                drain_epi()
            nc.sync.dma_start(out=y[:], in_=yt[:])

    nc.compile()
    return nc


def _host_prep(x, weight, bias, w_mask, b_mask):
    """Shard + lay out inputs for the 8 cores. Layout-only (plus exact *2)."""
    import ml_dtypes

    x = np.ascontiguousarray(x, dtype=np.float32)
    weight = np.ascontiguousarray(weight, dtype=np.float32)
    bias = np.ascontiguousarray(bias, dtype=np.float32)
    b_mask = np.ascontiguousarray(b_mask, dtype=np.float32)

    wp = (2.0 * weight).reshape(P, F).astype(ml_dtypes.bfloat16)  # k = 8p + j
    bias2d = (2.0 * bias).reshape(OT, P)

    in_maps = []
    for c in range(N_CORES):
        sl = slice(c * NS, (c + 1) * NS)
        wm_c = w_mask[sl].reshape(NS, P, F)
        xt_c = np.ascontiguousarray(
            x[sl].T.reshape(P, J, NS).reshape(P, J * NS)
        ).astype(ml_dtypes.bfloat16)                  # col = j*NS + n
        bm = b_mask[sl].reshape(NS, OT, P)
        # bb[p, n*OT + ot] = 2*bias[ot*P+p] * b_mask[n, ot*P+p]
        bb_c = np.ascontiguousarray(
            (bias2d[None, :, :] * bm).transpose(2, 0, 1).reshape(P, NS * OT)
        ).astype(np.float32)
        in_maps.append(
            {
                "wm16": np.ascontiguousarray(wm_c[:, :, 0:C0]).astype(
                    ml_dtypes.bfloat16
                ),
                "wm8": np.ascontiguousarray(wm_c[:, :, C0:]).astype(
                    ml_dtypes.float8_e4m3fn
                ),
                "wp": wp,
                "xt": xt_c,
                "bb": bb_c,
            }
        )
    return in_maps


def _unshard(res):
    outs = []
    for c in range(N_CORES):
        v = res.results[c]["y"].reshape(P, NS, OT)
        # y_core[n, ot*P + p] = v[p, n, ot]
        outs.append(v.transpose(1, 2, 0).reshape(NS, D))
    return np.concatenate(outs, axis=0)


def kernel(x, weight, bias, w_mask, b_mask):
    # accept jax or numpy arrays
    x, weight, bias, w_mask, b_mask = (
        np.asarray(a) for a in (x, weight, bias, w_mask, b_mask)
    )
    in_maps = _host_prep(x, weight, bias, w_mask, b_mask)
    nc = _build_nc()
    res = run_bass_kernel_spmd(
        nc,
        in_maps,
        core_ids=list(range(N_CORES)),
        trace=TRACE["trace"],
        **TRACE["trace_kwargs"],
    )
    TRACE["last_result"] = res
    return _unshard(res).astype(np.float32, copy=False)


def sim_fill(sim, x, weight, bias, w_mask, b_mask, ns):
    """Fill CoreSim dram tensors for a reduced-ns structural check."""
    import ml_dtypes

    wm = w_mask.reshape(ns, P, F)
    sim.tensor("wm16")[:] = wm[:, :, 0:C0].astype(ml_dtypes.bfloat16)
    sim.tensor("wm8")[:] = wm[:, :, C0:].astype(ml_dtypes.float8_e4m3)
    sim.tensor("wp")[:] = (
        (2.0 * weight).reshape(P, F).astype(ml_dtypes.bfloat16)
    )
    sim.tensor("xt")[:] = (
        np.ascontiguousarray(x.T.reshape(P, J, ns).reshape(P, J * ns))
        .astype(ml_dtypes.bfloat16)
    )
    bias2d = (2.0 * bias).reshape(OT, P)
    bm = b_mask.reshape(ns, OT, P)
    sim.tensor("bb")[:] = (
        (bias2d[None, :, :] * bm).transpose(2, 0, 1).reshape(P, ns * OT)
    ).astype(np.float32)


def sim_read_y(sim, ns):
    v = np.array(sim.tensor("y")).reshape(P, ns, OT)
    return v.transpose(1, 2, 0).reshape(ns, D)
